# revision 20
# baseline (speedup 1.0000x reference)
"""DenseEnergyLoss Bass kernel for TRN2, 8-core data parallel (2 images/core).

loss = -1e-7/N * sum_p (gate[p]/den[p]) * sum_o w[o,p] * C[o,p]
  C[o,p] = sum_k s[k,p] * s[k,p+off_o],  s = seg_roi (downsampled segs * roi)
  w[o,p] = sw_o * exp(-(L1_color_diff)^2 / (2*15^2)), guide = downsampled image (/255)
Offsets: circle radius 7 (149 taps) folded to 75 by pair symmetry; the
row-shifted partner accumulations bounce through DRAM planes because compute
engines cannot read/write partition-shifted operands.
"""
import sys
sys.path.insert(0, '/opt/trn_rl_repo')
import numpy as np

WEIGHT = 1e-07
SIGMA_RGB = 15.0
SIGMA_XY = 100.0
SCALE = 0.5
RADIUS = 7
N, C, H, W, K = 16, 3, 320, 320, 21
NCORES = 8
NIMG = N // NCORES           # 2 images per core
HS, WS = H // 2, W // 2      # 160
PADW = WS + 2 * RADIUS       # 174
PADH = HS + 2 * RADIUS       # 174 padded rows per image
TR = NIMG * PADH             # 348 stacked padded rows
SLABS = [(0, 128), (128, 128), (256, TR - 256)]   # (base, nrows)

INV2SC2 = 1.0 / (2.0 * SIGMA_RGB ** 2)


def _offsets_half():
    """Half-space offsets: (0,0) + {di=0,dj>0} + {di>0}, with r2<=R^2."""
    offs = []
    for di in range(0, RADIUS + 1):
        for dj in range(-RADIUS, RADIUS + 1):
            if di * di + dj * dj > RADIUS * RADIUS:
                continue
            if di == 0 and dj < 0:
                continue
            offs.append((di, dj))
    return offs

def _grouped_offsets():
    """Group half-space offsets by dj (same col window), chunks of <= GMAX."""
    byd = {}
    for di, dj in _offsets_half():
        byd.setdefault(dj, []).append(di)
    groups = []
    flat = []
    for dj in sorted(byd):
        dis = sorted(byd[dj])
        for c0 in range(0, len(dis), GMAX):
            chunk = dis[c0:c0 + GMAX]
            groups.append((dj, chunk, len(flat)))
            flat.extend((di, dj) for di in chunk)
    return groups, flat

GMAX = 3
GROUPS, OFFS = _grouped_offsets()   # 75 offsets incl (0,0), grouped by dj
import math as _math
SWV = [ _math.exp(-(di * di + dj * dj) / (2.0 * (SIGMA_XY * SCALE) ** 2)) for di, dj in OFFS ]


def _reflect(t):
    # jnp.pad 'reflect' index mapping on [0, HS)
    if t < 0:
        return -t
    if t > HS - 1:
        return 2 * (HS - 1) - t
    return t


def _row_runs(base, nrows):
    """Decompose slab partitions into runs of (p0, n, img, ir0, step) where
    stacked row base+p maps to image `img`, image row ir (downsampled grid)."""
    runs = []
    p = 0
    while p < nrows:
        sr = base + p
        img, pr = sr // PADH, sr % PADH
        ir = _reflect(pr - RADIUS)
        # step direction of ir as pr increases
        if pr - RADIUS < 0:
            step = -1
            n = min(nrows - p, RADIUS - pr)  # pr in [0, RADIUS)
        elif pr - RADIUS > HS - 1:
            step = -1
            n = min(nrows - p, PADH - pr)
        else:
            step = 1
            n = min(nrows - p, (HS - 1) - (pr - RADIUS) + 1, PADH - pr)
        runs.append((p, n, img, ir, step))
        p += n
    return runs


def build_bass(repeat=1):
    import concourse.bacc as bacc
    import concourse.tile as tile
    from concourse import mybir
    from concourse import bass_isa

    f32 = mybir.dt.float32
    bf16 = mybir.dt.bfloat16
    i32 = mybir.dt.int32
    Alu = mybir.AluOpType
    ActF = mybir.ActivationFunctionType
    AX = mybir.AxisListType

    nc = bacc.Bacc("TRN2", target_bir_lowering=False, debug=False)

    # ---- I/O ----
    d_img = nc.dram_tensor("images", [NIMG, C, H, W], f32, kind="ExternalInput").ap()
    d_seg = nc.dram_tensor("segmentations", [NIMG, K, H, W], f32, kind="ExternalInput").ap()
    d_roi = nc.dram_tensor("ROIs", [NIMG, H, W], f32, kind="ExternalInput").ap()
    d_lab = nc.dram_tensor("seg_label", [NIMG, H, W], i32, kind="ExternalInput").ap()
    # host-prepared constants
    d_lnsw = nc.dram_tensor("lnsw", [1, len(OFFS)], f32, kind="ExternalInput").ap()
    d_rowimg = nc.dram_tensor("rowimg", [len(SLABS), 128], f32, kind="ExternalInput").ap()
    d_vmask = nc.dram_tensor("vmask", [len(SLABS), 128], f32, kind="ExternalInput").ap()
    d_out = nc.dram_tensor("out", [128], f32, kind="ExternalOutput").ap()

    # ---- DRAM scratch ----
    s_pad = nc.dram_tensor("s_pad", [TR, K, PADW], bf16).ap()
    g_pad = nc.dram_tensor("g_pad", [TR, C, PADW], bf16).ap()
    gd_pad = nc.dram_tensor("gd_pad", [TR, PADW], f32).ap()
    q_n = nc.dram_tensor("q_n", [RADIUS, TR, PADW], f32).ap()   # partner numl planes, di-1 indexed
    q_d = nc.dram_tensor("q_d", [RADIUS, TR, PADW], f32).ap()   # partner den planes
    gm_d = nc.dram_tensor("gm_d", [len(SLABS), 128], f32).ap()  # per-slab per-partition guide maxes

    with tile.TileContext(nc) as tc:
      for _rep in range(repeat):
        # ================= Phase A: downsample + scratch build =================
        with tc.tile_pool(name="pa", bufs=1) as pa, tc.tile_pool(name="pa1", bufs=1) as pa1:
            zt = pa1.tile([128, PADW], f32, tag="zeros")
            nc.gpsimd.memset(zt[:], 0.0)
            for si, (base, nrows) in enumerate(SLABS):
                runs = _row_runs(base, nrows)
                araw = pa.tile([128, K, 2, W], f32, tag="araw")
                graw = pa.tile([128, C, WS], f32, tag="graw")
                rraw = pa.tile([128, WS], f32, tag="rraw")
                lraw = pa.tile([128, WS], i32, tag="lraw")
                for (p0, n, img, ir0, step) in runs:
                    if step > 0:
                        # seg raw rows 2*ir and 2*ir+1 -> [n, K, 2, W]; one DMA per parity
                        nc.sync.dma_start(
                            araw[p0:p0 + n, :, 0, :],
                            d_seg[img, :, 2 * ir0: 2 * (ir0 + n): 2, :].rearrange("k r w -> r k w"))
                        nc.sync.dma_start(
                            araw[p0:p0 + n, :, 1, :],
                            d_seg[img, :, 2 * ir0 + 1: 2 * (ir0 + n): 2, :].rearrange("k r w -> r k w"))
                        imgsrc = d_img[img, :, 2 * ir0: 2 * (ir0 + n): 2, 0:W:2]
                        for ch in range(C):
                            nc.sync.dma_start(graw[p0:p0 + n, ch, :], imgsrc[ch])
                        nc.sync.dma_start(rraw[p0:p0 + n, :], d_roi[img, 2 * ir0: 2 * (ir0 + n): 2, 0:W:2])
                        nc.sync.dma_start(lraw[p0:p0 + n, :], d_lab[img, 2 * ir0: 2 * (ir0 + n): 2, 0:W:2])
                    else:
                        # reflect run: descending image rows; HW forbids negative
                        # partition steps in DMA, so emit per-row DMAs
                        for t in range(n):
                            ir = ir0 - t
                            p = p0 + t
                            nc.sync.dma_start(
                                araw[p:p + 1, :, :, :],
                                d_seg[img, :, 2 * ir: 2 * ir + 2, :].unsqueeze(0))
                            nc.sync.dma_start(graw[p:p + 1, :, :], d_img[img, :, 2 * ir, 0:W:2].unsqueeze(0))
                            nc.sync.dma_start(rraw[p:p + 1, :], d_roi[img, 2 * ir, 0:W:2].unsqueeze(0))
                            nc.sync.dma_start(lraw[p:p + 1, :], d_lab[img, 2 * ir, 0:W:2].unsqueeze(0))

                # downsample: b1 = row pair sum [nr, K, W]; b2 = col pair sum [nr, K, WS]
                nr = nrows
                b1 = pa.tile([128, K, W], f32, tag="b1")
                nc.vector.tensor_tensor(out=b1[0:nr], in0=araw[0:nr, :, 0, :], in1=araw[0:nr, :, 1, :], op=Alu.add)
                b2 = pa.tile([128, K, WS], f32, tag="b2")
                nc.vector.tensor_tensor(out=b2[0:nr], in0=b1[0:nr, :, 0:W:2], in1=b1[0:nr, :, 1:W:2], op=Alu.add)
                # segmax (pre-roi, = 4*seg_s max)
                smax = pa.tile([128, WS], f32, tag="smax")
                nc.vector.tensor_reduce(smax[0:nr], b2[0:nr].transpose([0, 2, 1]), AX.X, Alu.max)
                # s = b2 * (roi * 0.25), bf16
                rq = pa.tile([128, WS], f32, tag="rq")
                nc.vector.tensor_scalar(out=rq[0:nr], in0=rraw[0:nr], scalar1=0.25, scalar2=None, op0=Alu.mult)
                sslab = pa.tile([128, K, PADW], bf16, tag="sslab")
                nc.vector.tensor_tensor(
                    out=sslab[0:nr, :, RADIUS:RADIUS + WS], in0=b2[0:nr],
                    in1=rq[0:nr].unsqueeze(1).to_broadcast([nr, K, WS]), op=Alu.mult)
                # column reflect pads
                nc.vector.tensor_copy(
                    out=sslab[0:nr, :, 0:RADIUS],
                    in_=sslab[0:nr, :, 2 * RADIUS:RADIUS:-1])
                nc.vector.tensor_copy(
                    out=sslab[0:nr, :, RADIUS + WS:PADW],
                    in_=sslab[0:nr, :, RADIUS + WS - 2:WS - 2:-1])
                nc.sync.dma_start(s_pad[base:base + nrows, :, :], sslab[0:nrows, :, :])

                # guide (raw 0..255 scale), bf16 + reflect pads
                gslab = pa.tile([128, C, PADW], bf16, tag="gslab")
                nc.vector.tensor_copy(out=gslab[0:nr, :, RADIUS:RADIUS + WS], in_=graw[0:nr])
                nc.vector.tensor_copy(
                    out=gslab[0:nr, :, 0:RADIUS], in_=gslab[0:nr, :, 2 * RADIUS:RADIUS:-1])
                nc.vector.tensor_copy(
                    out=gslab[0:nr, :, RADIUS + WS:PADW],
                    in_=gslab[0:nr, :, RADIUS + WS - 2:WS - 2:-1])
                nc.sync.dma_start(g_pad[base:base + nrows, :, :], gslab[0:nrows, :, :])

                # per-partition guide max -> gm_d
                gmx = pa.tile([128, 1], f32, tag="gmx")
                nc.vector.tensor_reduce(gmx[0:nr], gslab[0:nr], AX.XY, Alu.max)
                nc.sync.dma_start(gm_d[si, 0:nrows], gmx[0:nrows, 0])

                # gate = unlab ? 1 : max(roi - smax*0.25, 0)
                un = pa.tile([128, WS], f32, tag="un")
                nc.vector.tensor_scalar(out=un[0:nr], in0=lraw[0:nr], scalar1=255, scalar2=None, op0=Alu.is_equal)
                g0 = pa.tile([128, WS], f32, tag="g0")
                nc.vector.tensor_scalar(out=g0[0:nr], in0=smax[0:nr], scalar1=-0.25, scalar2=None, op0=Alu.mult)
                nc.vector.tensor_tensor(out=g0[0:nr], in0=g0[0:nr], in1=rraw[0:nr], op=Alu.add)
                nc.vector.tensor_scalar(out=g0[0:nr], in0=g0[0:nr], scalar1=0.0, scalar2=None, op0=Alu.max)
                um1 = pa.tile([128, WS], f32, tag="um1")
                nc.vector.tensor_scalar(out=um1[0:nr], in0=un[0:nr], scalar1=-1.0, scalar2=1.0, op0=Alu.mult, op1=Alu.add)
                nc.vector.tensor_tensor(out=g0[0:nr], in0=g0[0:nr], in1=um1[0:nr], op=Alu.mult)
                nc.vector.tensor_tensor(out=g0[0:nr], in0=g0[0:nr], in1=un[0:nr], op=Alu.add)
                # gate plane: zero everything, then write real rows' interior cols
                nc.sync.dma_start(gd_pad[base:base + nrows, :], zt[0:nrows, :])
                for (p0, n, img, ir0, step) in runs:
                    pr0 = (base + p0) % PADH
                    if RADIUS <= pr0 < RADIUS + HS and step > 0:  # real interior run
                        nc.sync.dma_start(
                            gd_pad[base + p0: base + p0 + n, RADIUS:RADIUS + WS],
                            g0[p0:p0 + n, :])
                # zero-init partner planes' leading rows (rows [0, di))
                if si == 0:
                    for di in range(1, RADIUS + 1):
                        nc.sync.dma_start(q_n[di - 1, 0:di, :], zt[0:di, :])
                        nc.sync.dma_start(q_d[di - 1, 0:di, :], zt[0:di, :])

        # ============ Phase A2: per-image scale from guide max ============
        with tc.tile_pool(name="ps", bufs=1) as ps:
            gmt = ps.tile([1, TR], f32, tag="gmt")
            nc.sync.dma_start(gmt[:], gm_d[:, :].rearrange("s p -> (s p)")[0:TR].unsqueeze(0))
            gmab = ps.tile([1, 2], f32, tag="gmab")
            nc.vector.tensor_reduce(gmab[:, 0:1], gmt[:, 0:PADH], AX.X, Alu.max)
            nc.vector.tensor_reduce(gmab[:, 1:2], gmt[:, PADH:2 * PADH], AX.X, Alu.max)
            # s_img = (gmax > 1) * c1 + c2 ; c2 = -inv2sc2, c1 = -inv2sc2*(1/255^2 - 1)
            c2 = -INV2SC2
            c1 = -INV2SC2 * (1.0 / (255.0 ** 2) - 1.0)
            scab = ps.tile([1, 2], f32, tag="scab")
            nc.vector.tensor_scalar(out=scab[:], in0=gmab[:], scalar1=1.0, scalar2=None, op0=Alu.is_gt)
            nc.vector.tensor_scalar(out=scab[:], in0=scab[:], scalar1=c1, scalar2=c2, op0=Alu.mult, op1=Alu.add)
            scab_b = ps.tile([128, 2], f32, tag="scab_b")
            nc.gpsimd.partition_broadcast(scab_b[:], scab[:])
            # per-slab scale columns: sA*(1-rowimg) + sB*rowimg
            scale_cols = []
            rit = ps.tile([128, len(SLABS)], f32, tag="rit")
            nc.sync.dma_start(rit[:], d_rowimg[:, :].rearrange("s p -> p s"))
            sct = ps.tile([128, len(SLABS)], f32, tag="sct")
            for si in range(len(SLABS)):
                # sct[:, si] = sA + (sB - sA) * rowimg  -> sA*(1-r)+sB*r
                dtmp = ps.tile([128, 1], f32, tag=f"dtmp{si}")
                nc.vector.tensor_tensor(out=dtmp[:], in0=scab_b[:, 1:2], in1=scab_b[:, 0:1], op=Alu.subtract)
                nc.vector.tensor_tensor(out=dtmp[:], in0=dtmp[:], in1=rit[:, si:si + 1], op=Alu.mult)
                nc.vector.tensor_tensor(out=sct[:, si:si + 1], in0=dtmp[:], in1=scab_b[:, 0:1], op=Alu.add)
            # lnsw broadcast table
            ln1 = ps.tile([1, len(OFFS)], f32, tag="ln1")
            nc.sync.dma_start(ln1[:], d_lnsw[:])
            lnt = ps.tile([128, len(OFFS)], f32, tag="lnt")
            nc.gpsimd.partition_broadcast(lnt[:], ln1[:])

            # ================= Phase B: stencil loop =================
            with tc.tile_pool(name="pb", bufs=1) as pb, \
                 tc.tile_pool(name="pacc", bufs=1) as pacc, \
                 tc.tile_pool(name="ptmp", bufs=2) as ptmp:
                accs = {}
                for si, (base, nrows) in enumerate(SLABS):
                    numl = pacc.tile([128, PADW], f32, tag=f"numl{si}")
                    den = pacc.tile([128, PADW], f32, tag=f"den{si}")
                    nc.gpsimd.memset(numl[:], 0.0)
                    nc.gpsimd.memset(den[:], 0.0)
                    pnd = {}
                    for di in range(1, RADIUS + 1):
                        pp = pacc.tile([128, 2, PADW], f32, tag=f"pnd{si}_{di}")
                        nc.gpsimd.memset(pp[:], 0.0)
                        pnd[di] = pp
                    accs[si] = (numl, den, pnd, None)

                    sTall = pb.tile([128, RADIUS + 1, K, PADW], bf16, tag="sTall")
                    gTall = pb.tile([128, RADIUS + 1, C, PADW], bf16, tag="gTall")
                    if nrows < 128 or base + RADIUS + 128 > TR:
                        nc.gpsimd.memset(sTall[:], 0.0)
                        nc.gpsimd.memset(gTall[:], 0.0)
                    for di in range(0, RADIUS + 1):
                        nr = min(128, TR - (base + di))
                        nc.sync.dma_start(sTall[0:nr, di, :, :], s_pad[base + di: base + di + nr, :, :])
                        nc.sync.dma_start(gTall[0:nr, di, :, :], g_pad[base + di: base + di + nr, :, :])

                    # prod batch buffer, memset once: full-width tree adds may
                    # read cols outside the current window (finite garbage, never
                    # consumed downstream)
                    prodb = pacc.tile([128, GMAX, K, PADW], bf16, tag="prodb")
                    nc.gpsimd.memset(prodb[:], 0.0)
                    for (dj, dis, o0) in GROUPS:
                        G = len(dis)
                        l = max(0, -dj)
                        ln = PADW - abs(dj)
                        sl0 = slice(l, l + ln)
                        sl1 = slice(l + dj, l + dj + ln)
                        # d = sum_ch |g0 - g_sh| per member (gpsimd), batched reduce
                        d0 = dis[0]
                        assert dis == list(range(d0, d0 + G)), (dj, dis)
                        dsub = ptmp.tile([128, GMAX, C, PADW], bf16, tag="dsub")
                        nc.gpsimd.tensor_tensor(
                            out=dsub[:, 0:G, :, sl0],
                            in0=gTall[:, 0, :, sl0].unsqueeze(1).to_broadcast([128, G, C, ln]),
                            in1=gTall[:, d0:d0 + G, :, sl1], op=Alu.subtract)
                        draw = ptmp.tile([128, GMAX, PADW], f32, tag="draw")
                        nc.vector.tensor_reduce(
                            draw[:, 0:G, sl0], dsub[:, 0:G, :, sl0].transpose([0, 1, 3, 2]),
                            AX.X, Alu.add, apply_absolute_value=True)
                        d2 = ptmp.tile([128, GMAX, PADW], f32, tag="d2")
                        nc.vector.tensor_tensor(
                            out=d2[:, 0:G, sl0], in0=draw[:, 0:G, sl0], in1=draw[:, 0:G, sl0],
                            op=Alu.mult)
                        # w = sw * exp(scale*d2): batched exp (bias 0), then sw mult
                        twb = ptmp.tile([128, 2, GMAX, PADW], f32, tag="twb")
                        wv = twb[:, 1]
                        nc.scalar.activation(
                            wv[:, 0:G, sl0], d2[:, 0:G, sl0], ActF.Exp,
                            bias=0.0, scale=sct[:, si:si + 1])
                        nc.vector.tensor_tensor(
                            out=wv[:, 0:G, sl0], in0=wv[:, 0:G, sl0],
                            in1=lnt[:, o0:o0 + G].unsqueeze(2).to_broadcast([128, G, ln]),
                            op=Alu.mult)
                        # C = sum_k s0 * s_sh, one batched mult per group
                        nc.vector.tensor_tensor(
                            out=prodb[:, 0:G, :, sl0],
                            in0=sTall[:, 0, :, sl0].unsqueeze(1).to_broadcast([128, G, K, ln]),
                            in1=sTall[:, d0:d0 + G, :, sl1], op=Alu.mult)
                        # batched bf16 pairwise k-reduce tree over full width
                        t10 = ptmp.tile([128, GMAX, 10, PADW], bf16, tag="t10")
                        nc.vector.tensor_tensor(
                            out=t10[:, 0:G], in0=prodb[:, 0:G, 0:10, :], in1=prodb[:, 0:G, 10:20, :], op=Alu.add)
                        t5 = ptmp.tile([128, GMAX, 5, PADW], bf16, tag="t5")
                        nc.vector.tensor_tensor(
                            out=t5[:, 0:G], in0=t10[:, 0:G, 0:5, :], in1=t10[:, 0:G, 5:10, :], op=Alu.add)
                        t2 = ptmp.tile([128, GMAX, 2, PADW], bf16, tag="t2")
                        nc.vector.tensor_tensor(
                            out=t2[:, 0:G], in0=t5[:, 0:G, 0:2, :], in1=t5[:, 0:G, 2:4, :], op=Alu.add)
                        t1 = ptmp.tile([128, GMAX, PADW], bf16, tag="t1")
                        nc.vector.tensor_tensor(
                            out=t1[:, 0:G], in0=t2[:, 0:G, 0, :], in1=t2[:, 0:G, 1, :], op=Alu.add)
                        nc.vector.tensor_tensor(
                            out=t1[:, 0:G], in0=t1[:, 0:G], in1=t5[:, 0:G, 4, :], op=Alu.add)
                        cred = ptmp.tile([128, GMAX, PADW], f32, tag="cred")
                        nc.vector.tensor_tensor(
                            out=cred[:, 0:G], in0=t1[:, 0:G], in1=prodb[:, 0:G, 20, :], op=Alu.add)
                        tv = twb[:, 0]
                        nc.gpsimd.tensor_tensor(
                            out=tv[:, 0:G, sl0], in0=wv[:, 0:G, sl0], in1=cred[:, 0:G, sl0], op=Alu.mult)
                        # own accumulation: numl += sum_g tv, den += sum_g wv
                        gsum = ptmp.tile([128, PADW], f32, tag="gsum")
                        nc.vector.tensor_reduce(
                            gsum[:, sl0], tv[:, 0:G, sl0].transpose([0, 2, 1]), AX.X, Alu.add)
                        nc.vector.tensor_tensor(out=numl[:, sl0], in0=numl[:, sl0], in1=gsum[:, sl0], op=Alu.add)
                        gsum2 = ptmp.tile([128, PADW], f32, tag="gsum2")
                        nc.vector.tensor_reduce(
                            gsum2[:, sl0], wv[:, 0:G, sl0].transpose([0, 2, 1]), AX.X, Alu.add)
                        nc.vector.tensor_tensor(out=den[:, sl0], in0=den[:, sl0], in1=gsum2[:, sl0], op=Alu.add)
                        # partner accumulation per member
                        for g, di in enumerate(dis):
                            if (di, dj) == (0, 0):
                                continue
                            if di == 0:
                                nc.vector.tensor_tensor(out=numl[:, sl1], in0=numl[:, sl1], in1=tv[:, g, sl0], op=Alu.add)
                                nc.vector.tensor_tensor(out=den[:, sl1], in0=den[:, sl1], in1=wv[:, g, sl0], op=Alu.add)
                            else:
                                pp = pnd[di]
                                nc.gpsimd.tensor_tensor(
                                    out=pp[:, :, sl1], in0=pp[:, :, sl1], in1=twb[:, :, g, sl0], op=Alu.add)

                    # write partner planes to DRAM at row offset +di
                    for di in range(1, RADIUS + 1):
                        nw = min(128, TR - (base + di))
                        nc.sync.dma_start(q_n[di - 1, base + di: base + di + nw, :], pnd[di][0:nw, 0, :])
                        nc.sync.dma_start(q_d[di - 1, base + di: base + di + nw, :], pnd[di][0:nw, 1, :])

                # ================= Phase C: finalize =================
                with tc.tile_pool(name="pc", bufs=2) as pc:
                    acc = ps.tile([128, 1], f32, tag="acc")
                    nc.gpsimd.memset(acc[:], 0.0)
                    vmt = ps.tile([128, len(SLABS)], f32, tag="vmt")
                    nc.sync.dma_start(vmt[:], d_vmask[:, :].rearrange("s p -> p s"))
                    for si, (base, nrows) in enumerate(SLABS):
                        numl, den, _, _ = accs[si]
                        for di in range(1, RADIUS + 1):
                            qn = pc.tile([128, PADW], f32, tag="qn")
                            qd = pc.tile([128, PADW], f32, tag="qd")
                            if nrows < 128:
                                nc.gpsimd.memset(qn[:], 0.0)
                                nc.gpsimd.memset(qd[:], 0.0)
                            nc.sync.dma_start(qn[0:nrows, :], q_n[di - 1, base:base + nrows, :])
                            nc.sync.dma_start(qd[0:nrows, :], q_d[di - 1, base:base + nrows, :])
                            nc.vector.tensor_tensor(out=numl[:], in0=numl[:], in1=qn[:], op=Alu.add)
                            nc.vector.tensor_tensor(out=den[:], in0=den[:], in1=qd[:], op=Alu.add)
                        gtile = pc.tile([128, PADW], f32, tag="gtile")
                        if nrows < 128:
                            nc.gpsimd.memset(gtile[:], 0.0)
                        nc.sync.dma_start(gtile[0:nrows, :], gd_pad[base:base + nrows, :])
                        rec = pc.tile([128, PADW], f32, tag="rec")
                        nc.vector.reciprocal(out=rec[:], in_=den[:])
                        nc.vector.tensor_tensor(out=rec[:], in0=rec[:], in1=gtile[:], op=Alu.mult)
                        nc.vector.tensor_tensor(out=rec[:], in0=rec[:], in1=numl[:], op=Alu.mult)
                        cs = pc.tile([128, 1], f32, tag="cs")
                        nc.vector.tensor_reduce(cs[:], rec[:], AX.X, Alu.add)
                        nc.vector.tensor_tensor(out=cs[:], in0=cs[:], in1=vmt[:, si:si + 1], op=Alu.mult)
                        nc.vector.tensor_tensor(out=acc[:], in0=acc[:], in1=cs[:], op=Alu.add)
                    nc.sync.dma_start(d_out[:], acc[:, 0])

    nc.compile()
    return nc


def host_consts():
    lnsw = np.array(SWV, dtype=np.float32)[None, :]
    rowimg = np.zeros((len(SLABS), 128), dtype=np.float32)
    vmask = np.zeros((len(SLABS), 128), dtype=np.float32)
    for si, (base, nrows) in enumerate(SLABS):
        for p in range(128):
            sr = base + p
            if sr < TR:
                rowimg[si, p] = float(sr // PADH)
                vmask[si, p] = 1.0
    return lnsw, rowimg, vmask


_NC_CACHE = {}


def get_nc(repeat=1):
    if repeat not in _NC_CACHE:
        _NC_CACHE[repeat] = build_bass(repeat)
    return _NC_CACHE[repeat]


def make_in_maps(images, segmentations, ROIs, seg_label):
    lnsw, rowimg, vmask = host_consts()
    in_maps = []
    for c in range(NCORES):
        sl = slice(c * NIMG, (c + 1) * NIMG)
        in_maps.append({
            "images": np.ascontiguousarray(images[sl], dtype=np.float32),
            "segmentations": np.ascontiguousarray(segmentations[sl], dtype=np.float32),
            "ROIs": np.ascontiguousarray(ROIs[sl], dtype=np.float32),
            "seg_label": np.ascontiguousarray(seg_label[sl, 0], dtype=np.int32),
            "lnsw": lnsw, "rowimg": rowimg, "vmask": vmask,
        })
    return in_maps


def kernel(images, segmentations, ROIs, seg_label):
    from concourse.bass_utils import run_bass_kernel_spmd
    nc = get_nc()
    in_maps = make_in_maps(images, segmentations, ROIs, seg_label)
    res = run_bass_kernel_spmd(nc, in_maps, list(range(NCORES)))
    total = 0.0
    for c in range(NCORES):
        total += float(np.sum(res.results[c]["out"].astype(np.float64)))
    loss = np.float32(-WEIGHT * total / N)
    return np.reshape(loss, (1,))


if __name__ == "__main__":
    rng = np.random.default_rng(0)
    imgs = rng.uniform(0, 255, (N, C, H, W)).astype(np.float32)
    segs = rng.standard_normal((N, K, H, W)).astype(np.float32)
    e = np.exp(segs - segs.max(axis=1, keepdims=True))
    segs = (e / e.sum(axis=1, keepdims=True)).astype(np.float32)
    rois = rng.integers(0, 2, (N, H, W)).astype(np.float32)
    labs = rng.integers(0, 256, (N, 1, H, W)).astype(np.int32)
    print(kernel(images=imgs, segmentations=segs, ROIs=rois, seg_label=labs))


# revision 21
# speedup vs baseline: 1.0024x; 1.0024x over previous
"""DenseEnergyLoss Bass kernel for TRN2, 8-core data parallel (2 images/core).

loss = -1e-7/N * sum_p (gate[p]/den[p]) * sum_o w[o,p] * C[o,p]
  C[o,p] = sum_k s[k,p] * s[k,p+off_o],  s = seg_roi (downsampled segs * roi)
  w[o,p] = sw_o * exp(-(L1_color_diff)^2 / (2*15^2)), guide = downsampled image (/255)
Offsets: circle radius 7 (149 taps) folded to 75 by pair symmetry; the
row-shifted partner accumulations bounce through DRAM planes because compute
engines cannot read/write partition-shifted operands.
"""
import sys
sys.path.insert(0, '/opt/trn_rl_repo')
import numpy as np

WEIGHT = 1e-07
SIGMA_RGB = 15.0
SIGMA_XY = 100.0
SCALE = 0.5
RADIUS = 7
N, C, H, W, K = 16, 3, 320, 320, 21
NCORES = 8
NIMG = N // NCORES           # 2 images per core
HS, WS = H // 2, W // 2      # 160
PADW = WS + 2 * RADIUS       # 174
PADH = HS + 2 * RADIUS       # 174 padded rows per image
TR = NIMG * PADH             # 348 stacked padded rows
SLABS = [(0, 128), (128, 128), (256, TR - 256)]   # (base, nrows)

INV2SC2 = 1.0 / (2.0 * SIGMA_RGB ** 2)


def _offsets_half():
    """Half-space offsets: (0,0) + {di=0,dj>0} + {di>0}, with r2<=R^2."""
    offs = []
    for di in range(0, RADIUS + 1):
        for dj in range(-RADIUS, RADIUS + 1):
            if di * di + dj * dj > RADIUS * RADIUS:
                continue
            if di == 0 and dj < 0:
                continue
            offs.append((di, dj))
    return offs

def _grouped_offsets():
    """Group half-space offsets by dj (same col window), chunks of <= GMAX."""
    byd = {}
    for di, dj in _offsets_half():
        byd.setdefault(dj, []).append(di)
    groups = []
    flat = []
    for dj in sorted(byd):
        dis = sorted(byd[dj])
        for c0 in range(0, len(dis), GMAX):
            chunk = dis[c0:c0 + GMAX]
            groups.append((dj, chunk, len(flat)))
            flat.extend((di, dj) for di in chunk)
    return groups, flat

GMAX = 3
GROUPS, OFFS = _grouped_offsets()   # 75 offsets incl (0,0), grouped by dj
import math as _math
SWV = [ _math.exp(-(di * di + dj * dj) / (2.0 * (SIGMA_XY * SCALE) ** 2)) for di, dj in OFFS ]


def _reflect(t):
    # jnp.pad 'reflect' index mapping on [0, HS)
    if t < 0:
        return -t
    if t > HS - 1:
        return 2 * (HS - 1) - t
    return t


def _row_runs(base, nrows):
    """Decompose slab partitions into runs of (p0, n, img, ir0, step) where
    stacked row base+p maps to image `img`, image row ir (downsampled grid)."""
    runs = []
    p = 0
    while p < nrows:
        sr = base + p
        img, pr = sr // PADH, sr % PADH
        ir = _reflect(pr - RADIUS)
        # step direction of ir as pr increases
        if pr - RADIUS < 0:
            step = -1
            n = min(nrows - p, RADIUS - pr)  # pr in [0, RADIUS)
        elif pr - RADIUS > HS - 1:
            step = -1
            n = min(nrows - p, PADH - pr)
        else:
            step = 1
            n = min(nrows - p, (HS - 1) - (pr - RADIUS) + 1, PADH - pr)
        runs.append((p, n, img, ir, step))
        p += n
    return runs


def build_bass(repeat=1):
    import concourse.bacc as bacc
    import concourse.tile as tile
    from concourse import mybir
    from concourse import bass_isa

    f32 = mybir.dt.float32
    bf16 = mybir.dt.bfloat16
    i32 = mybir.dt.int32
    Alu = mybir.AluOpType
    ActF = mybir.ActivationFunctionType
    AX = mybir.AxisListType

    nc = bacc.Bacc("TRN2", target_bir_lowering=False, debug=False)

    # ---- I/O ----
    d_img = nc.dram_tensor("images", [NIMG, C, H, W], f32, kind="ExternalInput").ap()
    d_seg = nc.dram_tensor("segmentations", [NIMG, K, H, W], f32, kind="ExternalInput").ap()
    d_roi = nc.dram_tensor("ROIs", [NIMG, H, W], f32, kind="ExternalInput").ap()
    d_lab = nc.dram_tensor("seg_label", [NIMG, H, W], i32, kind="ExternalInput").ap()
    # host-prepared constants
    d_lnsw = nc.dram_tensor("lnsw", [1, len(OFFS)], f32, kind="ExternalInput").ap()
    d_rowimg = nc.dram_tensor("rowimg", [len(SLABS), 128], f32, kind="ExternalInput").ap()
    d_vmask = nc.dram_tensor("vmask", [len(SLABS), 128], f32, kind="ExternalInput").ap()
    d_out = nc.dram_tensor("out", [128], f32, kind="ExternalOutput").ap()

    # ---- DRAM scratch ----
    s_pad = nc.dram_tensor("s_pad", [TR, K, PADW], bf16).ap()
    g_pad = nc.dram_tensor("g_pad", [TR, C, PADW], bf16).ap()
    gd_pad = nc.dram_tensor("gd_pad", [TR, PADW], f32).ap()
    q_n = nc.dram_tensor("q_n", [RADIUS, TR, PADW], f32).ap()   # partner numl planes, di-1 indexed
    q_d = nc.dram_tensor("q_d", [RADIUS, TR, PADW], f32).ap()   # partner den planes
    gm_d = nc.dram_tensor("gm_d", [len(SLABS), 128], f32).ap()  # per-slab per-partition guide maxes

    with tile.TileContext(nc) as tc:
      for _rep in range(repeat):
        # ================= Phase A: downsample + scratch build =================
        with tc.tile_pool(name="pa", bufs=1) as pa, tc.tile_pool(name="pa1", bufs=1) as pa1:
            zt = pa1.tile([128, PADW], f32, tag="zeros")
            nc.gpsimd.memset(zt[:], 0.0)
            for si, (base, nrows) in enumerate(SLABS):
                runs = _row_runs(base, nrows)
                araw = pa.tile([128, K, 2, W], f32, tag="araw")
                graw = pa.tile([128, C, WS], f32, tag="graw")
                rraw = pa.tile([128, WS], f32, tag="rraw")
                lraw = pa.tile([128, WS], i32, tag="lraw")
                for (p0, n, img, ir0, step) in runs:
                    if step > 0:
                        # seg raw rows 2*ir and 2*ir+1 -> [n, K, 2, W]; one DMA per parity
                        nc.sync.dma_start(
                            araw[p0:p0 + n, :, 0, :],
                            d_seg[img, :, 2 * ir0: 2 * (ir0 + n): 2, :].rearrange("k r w -> r k w"))
                        nc.sync.dma_start(
                            araw[p0:p0 + n, :, 1, :],
                            d_seg[img, :, 2 * ir0 + 1: 2 * (ir0 + n): 2, :].rearrange("k r w -> r k w"))
                        imgsrc = d_img[img, :, 2 * ir0: 2 * (ir0 + n): 2, 0:W:2]
                        for ch in range(C):
                            nc.sync.dma_start(graw[p0:p0 + n, ch, :], imgsrc[ch])
                        nc.sync.dma_start(rraw[p0:p0 + n, :], d_roi[img, 2 * ir0: 2 * (ir0 + n): 2, 0:W:2])
                        nc.sync.dma_start(lraw[p0:p0 + n, :], d_lab[img, 2 * ir0: 2 * (ir0 + n): 2, 0:W:2])
                    else:
                        # reflect run: descending image rows; HW forbids negative
                        # partition steps in DMA, so emit per-row DMAs
                        for t in range(n):
                            ir = ir0 - t
                            p = p0 + t
                            nc.sync.dma_start(
                                araw[p:p + 1, :, :, :],
                                d_seg[img, :, 2 * ir: 2 * ir + 2, :].unsqueeze(0))
                            nc.sync.dma_start(graw[p:p + 1, :, :], d_img[img, :, 2 * ir, 0:W:2].unsqueeze(0))
                            nc.sync.dma_start(rraw[p:p + 1, :], d_roi[img, 2 * ir, 0:W:2].unsqueeze(0))
                            nc.sync.dma_start(lraw[p:p + 1, :], d_lab[img, 2 * ir, 0:W:2].unsqueeze(0))

                # downsample: b1 = row pair sum [nr, K, W]; b2 = col pair sum [nr, K, WS]
                nr = nrows
                b1 = pa.tile([128, K, W], f32, tag="b1")
                nc.vector.tensor_tensor(out=b1[0:nr], in0=araw[0:nr, :, 0, :], in1=araw[0:nr, :, 1, :], op=Alu.add)
                b2 = pa.tile([128, K, WS], f32, tag="b2")
                nc.vector.tensor_tensor(out=b2[0:nr], in0=b1[0:nr, :, 0:W:2], in1=b1[0:nr, :, 1:W:2], op=Alu.add)
                # segmax (pre-roi, = 4*seg_s max)
                smax = pa.tile([128, WS], f32, tag="smax")
                nc.vector.tensor_reduce(smax[0:nr], b2[0:nr].transpose([0, 2, 1]), AX.X, Alu.max)
                # s = b2 * (roi * 0.25), bf16
                rq = pa.tile([128, WS], f32, tag="rq")
                nc.vector.tensor_scalar(out=rq[0:nr], in0=rraw[0:nr], scalar1=0.25, scalar2=None, op0=Alu.mult)
                sslab = pa.tile([128, K, PADW], bf16, tag="sslab")
                nc.vector.tensor_tensor(
                    out=sslab[0:nr, :, RADIUS:RADIUS + WS], in0=b2[0:nr],
                    in1=rq[0:nr].unsqueeze(1).to_broadcast([nr, K, WS]), op=Alu.mult)
                # column reflect pads
                nc.vector.tensor_copy(
                    out=sslab[0:nr, :, 0:RADIUS],
                    in_=sslab[0:nr, :, 2 * RADIUS:RADIUS:-1])
                nc.vector.tensor_copy(
                    out=sslab[0:nr, :, RADIUS + WS:PADW],
                    in_=sslab[0:nr, :, RADIUS + WS - 2:WS - 2:-1])
                nc.sync.dma_start(s_pad[base:base + nrows, :, :], sslab[0:nrows, :, :])

                # guide (raw 0..255 scale), bf16 + reflect pads
                gslab = pa.tile([128, C, PADW], bf16, tag="gslab")
                nc.vector.tensor_copy(out=gslab[0:nr, :, RADIUS:RADIUS + WS], in_=graw[0:nr])
                nc.vector.tensor_copy(
                    out=gslab[0:nr, :, 0:RADIUS], in_=gslab[0:nr, :, 2 * RADIUS:RADIUS:-1])
                nc.vector.tensor_copy(
                    out=gslab[0:nr, :, RADIUS + WS:PADW],
                    in_=gslab[0:nr, :, RADIUS + WS - 2:WS - 2:-1])
                nc.sync.dma_start(g_pad[base:base + nrows, :, :], gslab[0:nrows, :, :])

                # per-partition guide max -> gm_d
                gmx = pa.tile([128, 1], f32, tag="gmx")
                nc.vector.tensor_reduce(gmx[0:nr], gslab[0:nr], AX.XY, Alu.max)
                nc.sync.dma_start(gm_d[si, 0:nrows], gmx[0:nrows, 0])

                # gate = unlab ? 1 : max(roi - smax*0.25, 0)
                un = pa.tile([128, WS], f32, tag="un")
                nc.vector.tensor_scalar(out=un[0:nr], in0=lraw[0:nr], scalar1=255, scalar2=None, op0=Alu.is_equal)
                g0 = pa.tile([128, WS], f32, tag="g0")
                nc.vector.tensor_scalar(out=g0[0:nr], in0=smax[0:nr], scalar1=-0.25, scalar2=None, op0=Alu.mult)
                nc.vector.tensor_tensor(out=g0[0:nr], in0=g0[0:nr], in1=rraw[0:nr], op=Alu.add)
                nc.vector.tensor_scalar(out=g0[0:nr], in0=g0[0:nr], scalar1=0.0, scalar2=None, op0=Alu.max)
                um1 = pa.tile([128, WS], f32, tag="um1")
                nc.vector.tensor_scalar(out=um1[0:nr], in0=un[0:nr], scalar1=-1.0, scalar2=1.0, op0=Alu.mult, op1=Alu.add)
                nc.vector.tensor_tensor(out=g0[0:nr], in0=g0[0:nr], in1=um1[0:nr], op=Alu.mult)
                nc.vector.tensor_tensor(out=g0[0:nr], in0=g0[0:nr], in1=un[0:nr], op=Alu.add)
                # gate plane: zero everything, then write real rows' interior cols
                nc.sync.dma_start(gd_pad[base:base + nrows, :], zt[0:nrows, :])
                for (p0, n, img, ir0, step) in runs:
                    pr0 = (base + p0) % PADH
                    if RADIUS <= pr0 < RADIUS + HS and step > 0:  # real interior run
                        nc.sync.dma_start(
                            gd_pad[base + p0: base + p0 + n, RADIUS:RADIUS + WS],
                            g0[p0:p0 + n, :])
                # zero-init partner planes' leading rows (rows [0, di))
                if si == 0:
                    for di in range(1, RADIUS + 1):
                        nc.sync.dma_start(q_n[di - 1, 0:di, :], zt[0:di, :])
                        nc.sync.dma_start(q_d[di - 1, 0:di, :], zt[0:di, :])

        # ============ Phase A2: per-image scale from guide max ============
        with tc.tile_pool(name="ps", bufs=1) as ps:
            gmt = ps.tile([1, TR], f32, tag="gmt")
            nc.sync.dma_start(gmt[:], gm_d[:, :].rearrange("s p -> (s p)")[0:TR].unsqueeze(0))
            gmab = ps.tile([1, 2], f32, tag="gmab")
            nc.vector.tensor_reduce(gmab[:, 0:1], gmt[:, 0:PADH], AX.X, Alu.max)
            nc.vector.tensor_reduce(gmab[:, 1:2], gmt[:, PADH:2 * PADH], AX.X, Alu.max)
            # s_img = (gmax > 1) * c1 + c2 ; c2 = -inv2sc2, c1 = -inv2sc2*(1/255^2 - 1)
            c2 = -INV2SC2
            c1 = -INV2SC2 * (1.0 / (255.0 ** 2) - 1.0)
            scab = ps.tile([1, 2], f32, tag="scab")
            nc.vector.tensor_scalar(out=scab[:], in0=gmab[:], scalar1=1.0, scalar2=None, op0=Alu.is_gt)
            nc.vector.tensor_scalar(out=scab[:], in0=scab[:], scalar1=c1, scalar2=c2, op0=Alu.mult, op1=Alu.add)
            scab_b = ps.tile([128, 2], f32, tag="scab_b")
            nc.gpsimd.partition_broadcast(scab_b[:], scab[:])
            # per-slab scale columns: sA*(1-rowimg) + sB*rowimg
            scale_cols = []
            rit = ps.tile([128, len(SLABS)], f32, tag="rit")
            nc.sync.dma_start(rit[:], d_rowimg[:, :].rearrange("s p -> p s"))
            sct = ps.tile([128, len(SLABS)], f32, tag="sct")
            for si in range(len(SLABS)):
                # sct[:, si] = sA + (sB - sA) * rowimg  -> sA*(1-r)+sB*r
                dtmp = ps.tile([128, 1], f32, tag=f"dtmp{si}")
                nc.vector.tensor_tensor(out=dtmp[:], in0=scab_b[:, 1:2], in1=scab_b[:, 0:1], op=Alu.subtract)
                nc.vector.tensor_tensor(out=dtmp[:], in0=dtmp[:], in1=rit[:, si:si + 1], op=Alu.mult)
                nc.vector.tensor_tensor(out=sct[:, si:si + 1], in0=dtmp[:], in1=scab_b[:, 0:1], op=Alu.add)
            # lnsw broadcast table
            ln1 = ps.tile([1, len(OFFS)], f32, tag="ln1")
            nc.sync.dma_start(ln1[:], d_lnsw[:])
            lnt = ps.tile([128, len(OFFS)], f32, tag="lnt")
            nc.gpsimd.partition_broadcast(lnt[:], ln1[:])

            # ================= Phase B: stencil loop =================
            with tc.tile_pool(name="pb", bufs=8) as pb, \
                 tc.tile_pool(name="pacc", bufs=1) as pacc, \
                 tc.tile_pool(name="ptmp", bufs=2) as ptmp:
                accs = {}
                for si, (base, nrows) in enumerate(SLABS):
                    numl = pacc.tile([128, PADW], f32, tag=f"numl{si}")
                    den = pacc.tile([128, PADW], f32, tag=f"den{si}")
                    nc.gpsimd.memset(numl[:], 0.0)
                    nc.gpsimd.memset(den[:], 0.0)
                    pns = {}
                    pds = {}
                    for di in range(1, RADIUS + 1):
                        pn = pacc.tile([128, PADW], f32, tag=f"pn{si}_{di}")
                        pd = pacc.tile([128, PADW], f32, tag=f"pd{si}_{di}")
                        nc.gpsimd.memset(pn[:], 0.0)
                        nc.gpsimd.memset(pd[:], 0.0)
                        pns[di], pds[di] = pn, pd
                    accs[si] = (numl, den, pns, pds)

                    sT = {}
                    gT = {}
                    for di in range(0, RADIUS + 1):
                        st = pb.tile([128, K, PADW], bf16, tag="sT")
                        gt = pb.tile([128, C, PADW], bf16, tag="gT")
                        nr = min(128, TR - (base + di))
                        if nr < 128:
                            nc.gpsimd.memset(st[:], 0.0)
                            nc.gpsimd.memset(gt[:], 0.0)
                        nc.sync.dma_start(st[0:nr, :, :], s_pad[base + di: base + di + nr, :, :])
                        nc.sync.dma_start(gt[0:nr, :, :], g_pad[base + di: base + di + nr, :, :])
                        sT[di], gT[di] = st, gt

                    # prod batch buffer, memset once: full-width tree adds may
                    # read cols outside the current window (finite garbage, never
                    # consumed downstream)
                    prodb = pacc.tile([128, GMAX, K, PADW], bf16, tag="prodb")
                    nc.gpsimd.memset(prodb[:], 0.0)
                    for (dj, dis, o0) in GROUPS:
                        G = len(dis)
                        l = max(0, -dj)
                        ln = PADW - abs(dj)
                        sl0 = slice(l, l + ln)
                        sl1 = slice(l + dj, l + dj + ln)
                        # d = sum_ch |g0 - g_sh| per member (gpsimd), batched reduce
                        dsub = ptmp.tile([128, GMAX, C, PADW], bf16, tag="dsub")
                        for g, di in enumerate(dis):
                            nc.gpsimd.tensor_tensor(
                                out=dsub[:, g, :, sl0], in0=gT[0][:, :, sl0],
                                in1=gT[di][:, :, sl1], op=Alu.subtract)
                        draw = ptmp.tile([128, GMAX, PADW], f32, tag="draw")
                        nc.vector.tensor_reduce(
                            draw[:, 0:G, sl0], dsub[:, 0:G, :, sl0].transpose([0, 1, 3, 2]),
                            AX.X, Alu.add, apply_absolute_value=True)
                        d2 = ptmp.tile([128, GMAX, PADW], f32, tag="d2")
                        nc.vector.tensor_tensor(
                            out=d2[:, 0:G, sl0], in0=draw[:, 0:G, sl0], in1=draw[:, 0:G, sl0],
                            op=Alu.mult)
                        # w = sw * exp(scale*d2): batched exp (bias 0), then sw mult
                        wv = ptmp.tile([128, GMAX, PADW], bf16, tag="wv")
                        nc.scalar.activation(
                            wv[:, 0:G, sl0], d2[:, 0:G, sl0], ActF.Exp,
                            bias=0.0, scale=sct[:, si:si + 1])
                        nc.vector.tensor_tensor(
                            out=wv[:, 0:G, sl0], in0=wv[:, 0:G, sl0],
                            in1=lnt[:, o0:o0 + G].unsqueeze(2).to_broadcast([128, G, ln]),
                            op=Alu.mult)
                        # C = sum_k s0 * s_sh, per member mult into batch planes
                        for g, di in enumerate(dis):
                            nc.vector.tensor_tensor(
                                out=prodb[:, g, :, sl0], in0=sT[0][:, :, sl0],
                                in1=sT[di][:, :, sl1], op=Alu.mult)
                        # batched bf16 pairwise k-reduce tree over full width
                        t10 = ptmp.tile([128, GMAX, 10, PADW], bf16, tag="t10")
                        nc.vector.tensor_tensor(
                            out=t10[:, 0:G], in0=prodb[:, 0:G, 0:10, :], in1=prodb[:, 0:G, 10:20, :], op=Alu.add)
                        t5 = ptmp.tile([128, GMAX, 5, PADW], bf16, tag="t5")
                        nc.vector.tensor_tensor(
                            out=t5[:, 0:G], in0=t10[:, 0:G, 0:5, :], in1=t10[:, 0:G, 5:10, :], op=Alu.add)
                        t2 = ptmp.tile([128, GMAX, 2, PADW], bf16, tag="t2")
                        nc.vector.tensor_tensor(
                            out=t2[:, 0:G], in0=t5[:, 0:G, 0:2, :], in1=t5[:, 0:G, 2:4, :], op=Alu.add)
                        t1 = ptmp.tile([128, GMAX, PADW], bf16, tag="t1")
                        nc.vector.tensor_tensor(
                            out=t1[:, 0:G], in0=t2[:, 0:G, 0, :], in1=t2[:, 0:G, 1, :], op=Alu.add)
                        nc.vector.tensor_tensor(
                            out=t1[:, 0:G], in0=t1[:, 0:G], in1=t5[:, 0:G, 4, :], op=Alu.add)
                        cred = ptmp.tile([128, GMAX, PADW], f32, tag="cred")
                        nc.vector.tensor_tensor(
                            out=cred[:, 0:G], in0=t1[:, 0:G], in1=prodb[:, 0:G, 20, :], op=Alu.add)
                        tv = ptmp.tile([128, GMAX, PADW], f32, tag="tv")
                        nc.gpsimd.tensor_tensor(
                            out=tv[:, 0:G, sl0], in0=wv[:, 0:G, sl0], in1=cred[:, 0:G, sl0], op=Alu.mult)
                        # own accumulation: numl += sum_g tv, den += sum_g wv
                        gsum = ptmp.tile([128, PADW], f32, tag="gsum")
                        nc.vector.tensor_reduce(
                            gsum[:, sl0], tv[:, 0:G, sl0].transpose([0, 2, 1]), AX.X, Alu.add)
                        nc.vector.tensor_tensor(out=numl[:, sl0], in0=numl[:, sl0], in1=gsum[:, sl0], op=Alu.add)
                        gsum2 = ptmp.tile([128, PADW], f32, tag="gsum2")
                        nc.vector.tensor_reduce(
                            gsum2[:, sl0], wv[:, 0:G, sl0].transpose([0, 2, 1]), AX.X, Alu.add)
                        nc.vector.tensor_tensor(out=den[:, sl0], in0=den[:, sl0], in1=gsum2[:, sl0], op=Alu.add)
                        # partner accumulation per member
                        for g, di in enumerate(dis):
                            if (di, dj) == (0, 0):
                                continue
                            if di == 0:
                                nc.vector.tensor_tensor(out=numl[:, sl1], in0=numl[:, sl1], in1=tv[:, g, sl0], op=Alu.add)
                                nc.vector.tensor_tensor(out=den[:, sl1], in0=den[:, sl1], in1=wv[:, g, sl0], op=Alu.add)
                            else:
                                pn, pd = pns[di], pds[di]
                                nc.gpsimd.tensor_tensor(out=pn[:, sl1], in0=pn[:, sl1], in1=tv[:, g, sl0], op=Alu.add)
                                nc.gpsimd.tensor_tensor(out=pd[:, sl1], in0=pd[:, sl1], in1=wv[:, g, sl0], op=Alu.add)

                    # write partner planes to DRAM at row offset +di
                    for di in range(1, RADIUS + 1):
                        nw = min(128, TR - (base + di))
                        nc.sync.dma_start(q_n[di - 1, base + di: base + di + nw, :], pns[di][0:nw, :])
                        nc.sync.dma_start(q_d[di - 1, base + di: base + di + nw, :], pds[di][0:nw, :])

                # ================= Phase C: finalize =================
                with tc.tile_pool(name="pc", bufs=2) as pc:
                    acc = ps.tile([128, 1], f32, tag="acc")
                    nc.gpsimd.memset(acc[:], 0.0)
                    vmt = ps.tile([128, len(SLABS)], f32, tag="vmt")
                    nc.sync.dma_start(vmt[:], d_vmask[:, :].rearrange("s p -> p s"))
                    for si, (base, nrows) in enumerate(SLABS):
                        numl, den, _, _ = accs[si]
                        for di in range(1, RADIUS + 1):
                            qn = pc.tile([128, PADW], f32, tag="qn")
                            qd = pc.tile([128, PADW], f32, tag="qd")
                            if nrows < 128:
                                nc.gpsimd.memset(qn[:], 0.0)
                                nc.gpsimd.memset(qd[:], 0.0)
                            nc.sync.dma_start(qn[0:nrows, :], q_n[di - 1, base:base + nrows, :])
                            nc.sync.dma_start(qd[0:nrows, :], q_d[di - 1, base:base + nrows, :])
                            nc.vector.tensor_tensor(out=numl[:], in0=numl[:], in1=qn[:], op=Alu.add)
                            nc.vector.tensor_tensor(out=den[:], in0=den[:], in1=qd[:], op=Alu.add)
                        gtile = pc.tile([128, PADW], f32, tag="gtile")
                        if nrows < 128:
                            nc.gpsimd.memset(gtile[:], 0.0)
                        nc.sync.dma_start(gtile[0:nrows, :], gd_pad[base:base + nrows, :])
                        rec = pc.tile([128, PADW], f32, tag="rec")
                        nc.vector.reciprocal(out=rec[:], in_=den[:])
                        nc.vector.tensor_tensor(out=rec[:], in0=rec[:], in1=gtile[:], op=Alu.mult)
                        nc.vector.tensor_tensor(out=rec[:], in0=rec[:], in1=numl[:], op=Alu.mult)
                        cs = pc.tile([128, 1], f32, tag="cs")
                        nc.vector.tensor_reduce(cs[:], rec[:], AX.X, Alu.add)
                        nc.vector.tensor_tensor(out=cs[:], in0=cs[:], in1=vmt[:, si:si + 1], op=Alu.mult)
                        nc.vector.tensor_tensor(out=acc[:], in0=acc[:], in1=cs[:], op=Alu.add)
                    nc.sync.dma_start(d_out[:], acc[:, 0])

    nc.compile()
    return nc


def host_consts():
    lnsw = np.array(SWV, dtype=np.float32)[None, :]
    rowimg = np.zeros((len(SLABS), 128), dtype=np.float32)
    vmask = np.zeros((len(SLABS), 128), dtype=np.float32)
    for si, (base, nrows) in enumerate(SLABS):
        for p in range(128):
            sr = base + p
            if sr < TR:
                rowimg[si, p] = float(sr // PADH)
                vmask[si, p] = 1.0
    return lnsw, rowimg, vmask


_NC_CACHE = {}


def get_nc(repeat=1):
    if repeat not in _NC_CACHE:
        _NC_CACHE[repeat] = build_bass(repeat)
    return _NC_CACHE[repeat]


def make_in_maps(images, segmentations, ROIs, seg_label):
    lnsw, rowimg, vmask = host_consts()
    in_maps = []
    for c in range(NCORES):
        sl = slice(c * NIMG, (c + 1) * NIMG)
        in_maps.append({
            "images": np.ascontiguousarray(images[sl], dtype=np.float32),
            "segmentations": np.ascontiguousarray(segmentations[sl], dtype=np.float32),
            "ROIs": np.ascontiguousarray(ROIs[sl], dtype=np.float32),
            "seg_label": np.ascontiguousarray(seg_label[sl, 0], dtype=np.int32),
            "lnsw": lnsw, "rowimg": rowimg, "vmask": vmask,
        })
    return in_maps


def kernel(images, segmentations, ROIs, seg_label):
    from concourse.bass_utils import run_bass_kernel_spmd
    nc = get_nc()
    in_maps = make_in_maps(images, segmentations, ROIs, seg_label)
    res = run_bass_kernel_spmd(nc, in_maps, list(range(NCORES)))
    total = 0.0
    for c in range(NCORES):
        total += float(np.sum(res.results[c]["out"].astype(np.float64)))
    loss = np.float32(-WEIGHT * total / N)
    return np.reshape(loss, (1,))


if __name__ == "__main__":
    rng = np.random.default_rng(0)
    imgs = rng.uniform(0, 255, (N, C, H, W)).astype(np.float32)
    segs = rng.standard_normal((N, K, H, W)).astype(np.float32)
    e = np.exp(segs - segs.max(axis=1, keepdims=True))
    segs = (e / e.sum(axis=1, keepdims=True)).astype(np.float32)
    rois = rng.integers(0, 2, (N, H, W)).astype(np.float32)
    labs = rng.integers(0, 256, (N, 1, H, W)).astype(np.int32)
    print(kernel(images=imgs, segmentations=segs, ROIs=rois, seg_label=labs))


# revision 22
# speedup vs baseline: 1.0273x; 1.0249x over previous
"""DenseEnergyLoss Bass kernel for TRN2, 8-core data parallel (2 images/core).

loss = -1e-7/N * sum_p (gate[p]/den[p]) * sum_o w[o,p] * C[o,p]
  C[o,p] = sum_k s[k,p] * s[k,p+off_o],  s = seg_roi (downsampled segs * roi)
  w[o,p] = sw_o * exp(-(L1_color_diff)^2 / (2*15^2)), guide = downsampled image (/255)
Offsets: circle radius 7 (149 taps) folded to 75 by pair symmetry; the
row-shifted partner accumulations bounce through DRAM planes because compute
engines cannot read/write partition-shifted operands.
"""
import sys
sys.path.insert(0, '/opt/trn_rl_repo')
import numpy as np

WEIGHT = 1e-07
SIGMA_RGB = 15.0
SIGMA_XY = 100.0
SCALE = 0.5
RADIUS = 7
N, C, H, W, K = 16, 3, 320, 320, 21
NCORES = 8
NIMG = N // NCORES           # 2 images per core
HS, WS = H // 2, W // 2      # 160
PADW = WS + 2 * RADIUS       # 174
PADH = HS + 2 * RADIUS       # 174 padded rows per image
TR = NIMG * PADH             # 348 stacked padded rows
SLABS = [(0, 128), (128, 128), (256, TR - 256)]   # (base, nrows)

INV2SC2 = 1.0 / (2.0 * SIGMA_RGB ** 2)


def _offsets_half():
    """Half-space offsets: (0,0) + {di=0,dj>0} + {di>0}, with r2<=R^2."""
    offs = []
    for di in range(0, RADIUS + 1):
        for dj in range(-RADIUS, RADIUS + 1):
            if di * di + dj * dj > RADIUS * RADIUS:
                continue
            if di == 0 and dj < 0:
                continue
            offs.append((di, dj))
    return offs

def _grouped_offsets():
    """Group half-space offsets by dj (same col window), chunks of <= GMAX."""
    byd = {}
    for di, dj in _offsets_half():
        byd.setdefault(dj, []).append(di)
    groups = []
    flat = []
    for dj in sorted(byd):
        dis = sorted(byd[dj])
        for c0 in range(0, len(dis), GMAX):
            chunk = dis[c0:c0 + GMAX]
            groups.append((dj, chunk, len(flat)))
            flat.extend((di, dj) for di in chunk)
    return groups, flat

GMAX = 3
GROUPS, OFFS = _grouped_offsets()   # 75 offsets incl (0,0), grouped by dj
import math as _math
SWV = [ _math.exp(-(di * di + dj * dj) / (2.0 * (SIGMA_XY * SCALE) ** 2)) for di, dj in OFFS ]


def _reflect(t):
    # jnp.pad 'reflect' index mapping on [0, HS)
    if t < 0:
        return -t
    if t > HS - 1:
        return 2 * (HS - 1) - t
    return t


def _row_runs(base, nrows):
    """Decompose slab partitions into runs of (p0, n, img, ir0, step) where
    stacked row base+p maps to image `img`, image row ir (downsampled grid)."""
    runs = []
    p = 0
    while p < nrows:
        sr = base + p
        img, pr = sr // PADH, sr % PADH
        ir = _reflect(pr - RADIUS)
        # step direction of ir as pr increases
        if pr - RADIUS < 0:
            step = -1
            n = min(nrows - p, RADIUS - pr)  # pr in [0, RADIUS)
        elif pr - RADIUS > HS - 1:
            step = -1
            n = min(nrows - p, PADH - pr)
        else:
            step = 1
            n = min(nrows - p, (HS - 1) - (pr - RADIUS) + 1, PADH - pr)
        runs.append((p, n, img, ir, step))
        p += n
    return runs


def build_bass(repeat=1):
    import concourse.bacc as bacc
    import concourse.tile as tile
    from concourse import mybir
    from concourse import bass_isa

    f32 = mybir.dt.float32
    bf16 = mybir.dt.bfloat16
    i32 = mybir.dt.int32
    Alu = mybir.AluOpType
    ActF = mybir.ActivationFunctionType
    AX = mybir.AxisListType

    nc = bacc.Bacc("TRN2", target_bir_lowering=False, debug=False)

    # ---- I/O ----
    d_img = nc.dram_tensor("images", [NIMG, C, H, W], f32, kind="ExternalInput").ap()
    d_seg = nc.dram_tensor("segmentations", [NIMG, K, H, W], f32, kind="ExternalInput").ap()
    d_roi = nc.dram_tensor("ROIs", [NIMG, H, W], f32, kind="ExternalInput").ap()
    d_lab = nc.dram_tensor("seg_label", [NIMG, H, W], i32, kind="ExternalInput").ap()
    # host-prepared constants
    d_lnsw = nc.dram_tensor("lnsw", [1, len(OFFS)], f32, kind="ExternalInput").ap()
    d_rowimg = nc.dram_tensor("rowimg", [len(SLABS), 128], f32, kind="ExternalInput").ap()
    d_vmask = nc.dram_tensor("vmask", [len(SLABS), 128], f32, kind="ExternalInput").ap()
    d_out = nc.dram_tensor("out", [128], f32, kind="ExternalOutput").ap()

    # ---- DRAM scratch ----
    s_pad = nc.dram_tensor("s_pad", [TR, K, PADW], bf16).ap()
    g_pad = nc.dram_tensor("g_pad", [TR, C, PADW], bf16).ap()
    gd_pad = nc.dram_tensor("gd_pad", [TR, PADW], f32).ap()
    q_n = nc.dram_tensor("q_n", [RADIUS, TR, PADW], f32).ap()   # partner numl planes, di-1 indexed
    q_d = nc.dram_tensor("q_d", [RADIUS, TR, PADW], f32).ap()   # partner den planes
    gm_d = nc.dram_tensor("gm_d", [len(SLABS), 128], f32).ap()  # per-slab per-partition guide maxes

    with tile.TileContext(nc) as tc:
      for _rep in range(repeat):
        # ================= Phase A: downsample + scratch build =================
        with tc.tile_pool(name="pa", bufs=1) as pa, tc.tile_pool(name="pa1", bufs=1) as pa1:
            zt = pa1.tile([128, PADW], f32, tag="zeros")
            nc.gpsimd.memset(zt[:], 0.0)
            for si, (base, nrows) in enumerate(SLABS):
                runs = _row_runs(base, nrows)
                araw = pa.tile([128, K, 2, W], f32, tag="araw")
                graw = pa.tile([128, C, WS], f32, tag="graw")
                rraw = pa.tile([128, WS], f32, tag="rraw")
                lraw = pa.tile([128, WS], i32, tag="lraw")
                for (p0, n, img, ir0, step) in runs:
                    if step > 0:
                        # seg raw rows 2*ir and 2*ir+1 -> [n, K, 2, W]; one DMA per parity
                        nc.sync.dma_start(
                            araw[p0:p0 + n, :, 0, :],
                            d_seg[img, :, 2 * ir0: 2 * (ir0 + n): 2, :].rearrange("k r w -> r k w"))
                        nc.sync.dma_start(
                            araw[p0:p0 + n, :, 1, :],
                            d_seg[img, :, 2 * ir0 + 1: 2 * (ir0 + n): 2, :].rearrange("k r w -> r k w"))
                        imgsrc = d_img[img, :, 2 * ir0: 2 * (ir0 + n): 2, 0:W:2]
                        for ch in range(C):
                            nc.sync.dma_start(graw[p0:p0 + n, ch, :], imgsrc[ch])
                        nc.sync.dma_start(rraw[p0:p0 + n, :], d_roi[img, 2 * ir0: 2 * (ir0 + n): 2, 0:W:2])
                        nc.sync.dma_start(lraw[p0:p0 + n, :], d_lab[img, 2 * ir0: 2 * (ir0 + n): 2, 0:W:2])
                    else:
                        # reflect run: descending image rows; HW forbids negative
                        # partition steps in DMA, so emit per-row DMAs
                        for t in range(n):
                            ir = ir0 - t
                            p = p0 + t
                            nc.sync.dma_start(
                                araw[p:p + 1, :, :, :],
                                d_seg[img, :, 2 * ir: 2 * ir + 2, :].unsqueeze(0))
                            nc.sync.dma_start(graw[p:p + 1, :, :], d_img[img, :, 2 * ir, 0:W:2].unsqueeze(0))
                            nc.sync.dma_start(rraw[p:p + 1, :], d_roi[img, 2 * ir, 0:W:2].unsqueeze(0))
                            nc.sync.dma_start(lraw[p:p + 1, :], d_lab[img, 2 * ir, 0:W:2].unsqueeze(0))

                # downsample: b1 = row pair sum [nr, K, W]; b2 = col pair sum [nr, K, WS]
                nr = nrows
                b1 = pa.tile([128, K, W], f32, tag="b1")
                nc.vector.tensor_tensor(out=b1[0:nr], in0=araw[0:nr, :, 0, :], in1=araw[0:nr, :, 1, :], op=Alu.add)
                b2 = pa.tile([128, K, WS], f32, tag="b2")
                nc.vector.tensor_tensor(out=b2[0:nr], in0=b1[0:nr, :, 0:W:2], in1=b1[0:nr, :, 1:W:2], op=Alu.add)
                # segmax (pre-roi, = 4*seg_s max)
                smax = pa.tile([128, WS], f32, tag="smax")
                nc.vector.tensor_reduce(smax[0:nr], b2[0:nr].transpose([0, 2, 1]), AX.X, Alu.max)
                # s = b2 * (roi * 0.25), bf16
                rq = pa.tile([128, WS], f32, tag="rq")
                nc.vector.tensor_scalar(out=rq[0:nr], in0=rraw[0:nr], scalar1=0.25, scalar2=None, op0=Alu.mult)
                sslab = pa.tile([128, K, PADW], bf16, tag="sslab")
                nc.vector.tensor_tensor(
                    out=sslab[0:nr, :, RADIUS:RADIUS + WS], in0=b2[0:nr],
                    in1=rq[0:nr].unsqueeze(1).to_broadcast([nr, K, WS]), op=Alu.mult)
                # column reflect pads
                nc.vector.tensor_copy(
                    out=sslab[0:nr, :, 0:RADIUS],
                    in_=sslab[0:nr, :, 2 * RADIUS:RADIUS:-1])
                nc.vector.tensor_copy(
                    out=sslab[0:nr, :, RADIUS + WS:PADW],
                    in_=sslab[0:nr, :, RADIUS + WS - 2:WS - 2:-1])
                nc.sync.dma_start(s_pad[base:base + nrows, :, :], sslab[0:nrows, :, :])

                # guide (raw 0..255 scale), bf16 + reflect pads
                gslab = pa.tile([128, C, PADW], bf16, tag="gslab")
                nc.vector.tensor_copy(out=gslab[0:nr, :, RADIUS:RADIUS + WS], in_=graw[0:nr])
                nc.vector.tensor_copy(
                    out=gslab[0:nr, :, 0:RADIUS], in_=gslab[0:nr, :, 2 * RADIUS:RADIUS:-1])
                nc.vector.tensor_copy(
                    out=gslab[0:nr, :, RADIUS + WS:PADW],
                    in_=gslab[0:nr, :, RADIUS + WS - 2:WS - 2:-1])
                nc.sync.dma_start(g_pad[base:base + nrows, :, :], gslab[0:nrows, :, :])

                # per-partition guide max -> gm_d
                gmx = pa.tile([128, 1], f32, tag="gmx")
                nc.vector.tensor_reduce(gmx[0:nr], gslab[0:nr], AX.XY, Alu.max)
                nc.sync.dma_start(gm_d[si, 0:nrows], gmx[0:nrows, 0])

                # gate = unlab ? 1 : max(roi - smax*0.25, 0)
                un = pa.tile([128, WS], f32, tag="un")
                nc.vector.tensor_scalar(out=un[0:nr], in0=lraw[0:nr], scalar1=255, scalar2=None, op0=Alu.is_equal)
                g0 = pa.tile([128, WS], f32, tag="g0")
                nc.vector.tensor_scalar(out=g0[0:nr], in0=smax[0:nr], scalar1=-0.25, scalar2=None, op0=Alu.mult)
                nc.vector.tensor_tensor(out=g0[0:nr], in0=g0[0:nr], in1=rraw[0:nr], op=Alu.add)
                nc.vector.tensor_scalar(out=g0[0:nr], in0=g0[0:nr], scalar1=0.0, scalar2=None, op0=Alu.max)
                um1 = pa.tile([128, WS], f32, tag="um1")
                nc.vector.tensor_scalar(out=um1[0:nr], in0=un[0:nr], scalar1=-1.0, scalar2=1.0, op0=Alu.mult, op1=Alu.add)
                nc.vector.tensor_tensor(out=g0[0:nr], in0=g0[0:nr], in1=um1[0:nr], op=Alu.mult)
                nc.vector.tensor_tensor(out=g0[0:nr], in0=g0[0:nr], in1=un[0:nr], op=Alu.add)
                # gate plane: zero everything, then write real rows' interior cols
                nc.sync.dma_start(gd_pad[base:base + nrows, :], zt[0:nrows, :])
                for (p0, n, img, ir0, step) in runs:
                    pr0 = (base + p0) % PADH
                    if RADIUS <= pr0 < RADIUS + HS and step > 0:  # real interior run
                        nc.sync.dma_start(
                            gd_pad[base + p0: base + p0 + n, RADIUS:RADIUS + WS],
                            g0[p0:p0 + n, :])
                # zero-init partner planes' leading rows (rows [0, di))
                if si == 0:
                    for di in range(1, RADIUS + 1):
                        nc.sync.dma_start(q_n[di - 1, 0:di, :], zt[0:di, :])
                        nc.sync.dma_start(q_d[di - 1, 0:di, :], zt[0:di, :])

        # ============ Phase A2: per-image scale from guide max ============
        with tc.tile_pool(name="ps", bufs=1) as ps:
            gmt = ps.tile([1, TR], f32, tag="gmt")
            nc.sync.dma_start(gmt[:], gm_d[:, :].rearrange("s p -> (s p)")[0:TR].unsqueeze(0))
            gmab = ps.tile([1, 2], f32, tag="gmab")
            nc.vector.tensor_reduce(gmab[:, 0:1], gmt[:, 0:PADH], AX.X, Alu.max)
            nc.vector.tensor_reduce(gmab[:, 1:2], gmt[:, PADH:2 * PADH], AX.X, Alu.max)
            # s_img = (gmax > 1) * c1 + c2 ; c2 = -inv2sc2, c1 = -inv2sc2*(1/255^2 - 1)
            c2 = -INV2SC2
            c1 = -INV2SC2 * (1.0 / (255.0 ** 2) - 1.0)
            scab = ps.tile([1, 2], f32, tag="scab")
            nc.vector.tensor_scalar(out=scab[:], in0=gmab[:], scalar1=1.0, scalar2=None, op0=Alu.is_gt)
            nc.vector.tensor_scalar(out=scab[:], in0=scab[:], scalar1=c1, scalar2=c2, op0=Alu.mult, op1=Alu.add)
            scab_b = ps.tile([128, 2], f32, tag="scab_b")
            nc.gpsimd.partition_broadcast(scab_b[:], scab[:])
            # per-slab scale columns: sA*(1-rowimg) + sB*rowimg
            scale_cols = []
            rit = ps.tile([128, len(SLABS)], f32, tag="rit")
            nc.sync.dma_start(rit[:], d_rowimg[:, :].rearrange("s p -> p s"))
            sct = ps.tile([128, len(SLABS)], f32, tag="sct")
            for si in range(len(SLABS)):
                # sct[:, si] = sA + (sB - sA) * rowimg  -> sA*(1-r)+sB*r
                dtmp = ps.tile([128, 1], f32, tag=f"dtmp{si}")
                nc.vector.tensor_tensor(out=dtmp[:], in0=scab_b[:, 1:2], in1=scab_b[:, 0:1], op=Alu.subtract)
                nc.vector.tensor_tensor(out=dtmp[:], in0=dtmp[:], in1=rit[:, si:si + 1], op=Alu.mult)
                nc.vector.tensor_tensor(out=sct[:, si:si + 1], in0=dtmp[:], in1=scab_b[:, 0:1], op=Alu.add)
            # lnsw broadcast table
            ln1 = ps.tile([1, len(OFFS)], f32, tag="ln1")
            nc.sync.dma_start(ln1[:], d_lnsw[:])
            lnt = ps.tile([128, len(OFFS)], f32, tag="lnt")
            nc.gpsimd.partition_broadcast(lnt[:], ln1[:])

            # ================= Phase B: stencil loop =================
            with tc.tile_pool(name="pb", bufs=1) as pb, \
                 tc.tile_pool(name="pacc", bufs=1) as pacc, \
                 tc.tile_pool(name="ptmp", bufs=2) as ptmp:
                accs = {}
                for si, (base, nrows) in enumerate(SLABS):
                    numl = pacc.tile([128, PADW], f32, tag=f"numl{si}")
                    den = pacc.tile([128, PADW], f32, tag=f"den{si}")
                    nc.gpsimd.memset(numl[:], 0.0)
                    nc.gpsimd.memset(den[:], 0.0)
                    pnd = {}
                    for di in range(1, RADIUS + 1):
                        pp = pacc.tile([128, 2, PADW], f32, tag=f"pnd{si}_{di}")
                        nc.gpsimd.memset(pp[:], 0.0)
                        pnd[di] = pp
                    accs[si] = (numl, den, pnd, None)

                    sTall = pb.tile([128, RADIUS + 1, K, PADW], bf16, tag="sTall")
                    gTall = pb.tile([128, RADIUS + 1, C, PADW], bf16, tag="gTall")
                    if nrows < 128 or base + RADIUS + 128 > TR:
                        nc.gpsimd.memset(sTall[:], 0.0)
                        nc.gpsimd.memset(gTall[:], 0.0)
                    for di in range(0, RADIUS + 1):
                        nr = min(128, TR - (base + di))
                        nc.sync.dma_start(sTall[0:nr, di, :, :], s_pad[base + di: base + di + nr, :, :])
                        nc.sync.dma_start(gTall[0:nr, di, :, :], g_pad[base + di: base + di + nr, :, :])

                    # prod batch buffer, memset once: full-width tree adds may
                    # read cols outside the current window (finite garbage, never
                    # consumed downstream)
                    prodb = pacc.tile([128, GMAX, K, PADW], bf16, tag="prodb")
                    nc.gpsimd.memset(prodb[:], 0.0)
                    for (dj, dis, o0) in GROUPS:
                        G = len(dis)
                        l = max(0, -dj)
                        ln = PADW - abs(dj)
                        sl0 = slice(l, l + ln)
                        sl1 = slice(l + dj, l + dj + ln)
                        # d = sum_ch |g0 - g_sh| per member (gpsimd), batched reduce
                        d0 = dis[0]
                        assert dis == list(range(d0, d0 + G)), (dj, dis)
                        dsub = ptmp.tile([128, GMAX, C, PADW], bf16, tag="dsub")
                        nc.gpsimd.tensor_tensor(
                            out=dsub[:, 0:G, :, sl0],
                            in0=gTall[:, 0, :, sl0].unsqueeze(1).to_broadcast([128, G, C, ln]),
                            in1=gTall[:, d0:d0 + G, :, sl1], op=Alu.subtract)
                        draw = ptmp.tile([128, GMAX, PADW], f32, tag="draw")
                        nc.vector.tensor_reduce(
                            draw[:, 0:G, sl0], dsub[:, 0:G, :, sl0].transpose([0, 1, 3, 2]),
                            AX.X, Alu.add, apply_absolute_value=True)
                        d2 = ptmp.tile([128, GMAX, PADW], f32, tag="d2")
                        nc.vector.tensor_tensor(
                            out=d2[:, 0:G, sl0], in0=draw[:, 0:G, sl0], in1=draw[:, 0:G, sl0],
                            op=Alu.mult)
                        # w = sw * exp(scale*d2): batched exp (bias 0), then sw mult
                        twb = ptmp.tile([128, 2, GMAX, PADW], f32, tag="twb")
                        wv = twb[:, 1]
                        nc.scalar.activation(
                            wv[:, 0:G, sl0], d2[:, 0:G, sl0], ActF.Exp,
                            bias=0.0, scale=sct[:, si:si + 1])
                        nc.vector.tensor_tensor(
                            out=wv[:, 0:G, sl0], in0=wv[:, 0:G, sl0],
                            in1=lnt[:, o0:o0 + G].unsqueeze(2).to_broadcast([128, G, ln]),
                            op=Alu.mult)
                        # C = sum_k s0 * s_sh, one batched mult per group
                        nc.vector.tensor_tensor(
                            out=prodb[:, 0:G, :, sl0],
                            in0=sTall[:, 0, :, sl0].unsqueeze(1).to_broadcast([128, G, K, ln]),
                            in1=sTall[:, d0:d0 + G, :, sl1], op=Alu.mult)
                        # batched bf16 pairwise k-reduce tree over full width
                        t10 = ptmp.tile([128, GMAX, 10, PADW], bf16, tag="t10")
                        nc.vector.tensor_tensor(
                            out=t10[:, 0:G], in0=prodb[:, 0:G, 0:10, :], in1=prodb[:, 0:G, 10:20, :], op=Alu.add)
                        t5 = ptmp.tile([128, GMAX, 5, PADW], bf16, tag="t5")
                        nc.vector.tensor_tensor(
                            out=t5[:, 0:G], in0=t10[:, 0:G, 0:5, :], in1=t10[:, 0:G, 5:10, :], op=Alu.add)
                        t2 = ptmp.tile([128, GMAX, 2, PADW], bf16, tag="t2")
                        nc.vector.tensor_tensor(
                            out=t2[:, 0:G], in0=t5[:, 0:G, 0:2, :], in1=t5[:, 0:G, 2:4, :], op=Alu.add)
                        t1 = ptmp.tile([128, GMAX, PADW], bf16, tag="t1")
                        nc.vector.tensor_tensor(
                            out=t1[:, 0:G], in0=t2[:, 0:G, 0, :], in1=t2[:, 0:G, 1, :], op=Alu.add)
                        nc.vector.tensor_tensor(
                            out=t1[:, 0:G], in0=t1[:, 0:G], in1=t5[:, 0:G, 4, :], op=Alu.add)
                        cred = ptmp.tile([128, GMAX, PADW], f32, tag="cred")
                        nc.vector.tensor_tensor(
                            out=cred[:, 0:G], in0=t1[:, 0:G], in1=prodb[:, 0:G, 20, :], op=Alu.add)
                        tv = twb[:, 0]
                        nc.gpsimd.tensor_tensor(
                            out=tv[:, 0:G, sl0], in0=wv[:, 0:G, sl0], in1=cred[:, 0:G, sl0], op=Alu.mult)
                        # own accumulation: numl += sum_g tv, den += sum_g wv
                        gsum = ptmp.tile([128, PADW], f32, tag="gsum")
                        nc.vector.tensor_reduce(
                            gsum[:, sl0], tv[:, 0:G, sl0].transpose([0, 2, 1]), AX.X, Alu.add)
                        nc.vector.tensor_tensor(out=numl[:, sl0], in0=numl[:, sl0], in1=gsum[:, sl0], op=Alu.add)
                        gsum2 = ptmp.tile([128, PADW], f32, tag="gsum2")
                        nc.vector.tensor_reduce(
                            gsum2[:, sl0], wv[:, 0:G, sl0].transpose([0, 2, 1]), AX.X, Alu.add)
                        nc.vector.tensor_tensor(out=den[:, sl0], in0=den[:, sl0], in1=gsum2[:, sl0], op=Alu.add)
                        # partner accumulation per member
                        for g, di in enumerate(dis):
                            if (di, dj) == (0, 0):
                                continue
                            if di == 0:
                                nc.vector.tensor_tensor(out=numl[:, sl1], in0=numl[:, sl1], in1=tv[:, g, sl0], op=Alu.add)
                                nc.vector.tensor_tensor(out=den[:, sl1], in0=den[:, sl1], in1=wv[:, g, sl0], op=Alu.add)
                            else:
                                pp = pnd[di]
                                nc.gpsimd.tensor_tensor(
                                    out=pp[:, :, sl1], in0=pp[:, :, sl1], in1=twb[:, :, g, sl0], op=Alu.add)

                    # write partner planes to DRAM at row offset +di
                    for di in range(1, RADIUS + 1):
                        nw = min(128, TR - (base + di))
                        nc.sync.dma_start(q_n[di - 1, base + di: base + di + nw, :], pnd[di][0:nw, 0, :])
                        nc.sync.dma_start(q_d[di - 1, base + di: base + di + nw, :], pnd[di][0:nw, 1, :])

                # ================= Phase C: finalize =================
                with tc.tile_pool(name="pc", bufs=2) as pc:
                    acc = ps.tile([128, 1], f32, tag="acc")
                    nc.gpsimd.memset(acc[:], 0.0)
                    vmt = ps.tile([128, len(SLABS)], f32, tag="vmt")
                    nc.sync.dma_start(vmt[:], d_vmask[:, :].rearrange("s p -> p s"))
                    for si, (base, nrows) in enumerate(SLABS):
                        numl, den, _, _ = accs[si]
                        for di in range(1, RADIUS + 1):
                            qn = pc.tile([128, PADW], f32, tag="qn")
                            qd = pc.tile([128, PADW], f32, tag="qd")
                            if nrows < 128:
                                nc.gpsimd.memset(qn[:], 0.0)
                                nc.gpsimd.memset(qd[:], 0.0)
                            nc.sync.dma_start(qn[0:nrows, :], q_n[di - 1, base:base + nrows, :])
                            nc.sync.dma_start(qd[0:nrows, :], q_d[di - 1, base:base + nrows, :])
                            nc.vector.tensor_tensor(out=numl[:], in0=numl[:], in1=qn[:], op=Alu.add)
                            nc.vector.tensor_tensor(out=den[:], in0=den[:], in1=qd[:], op=Alu.add)
                        gtile = pc.tile([128, PADW], f32, tag="gtile")
                        if nrows < 128:
                            nc.gpsimd.memset(gtile[:], 0.0)
                        nc.sync.dma_start(gtile[0:nrows, :], gd_pad[base:base + nrows, :])
                        rec = pc.tile([128, PADW], f32, tag="rec")
                        nc.vector.reciprocal(out=rec[:], in_=den[:])
                        nc.vector.tensor_tensor(out=rec[:], in0=rec[:], in1=gtile[:], op=Alu.mult)
                        nc.vector.tensor_tensor(out=rec[:], in0=rec[:], in1=numl[:], op=Alu.mult)
                        cs = pc.tile([128, 1], f32, tag="cs")
                        nc.vector.tensor_reduce(cs[:], rec[:], AX.X, Alu.add)
                        nc.vector.tensor_tensor(out=cs[:], in0=cs[:], in1=vmt[:, si:si + 1], op=Alu.mult)
                        nc.vector.tensor_tensor(out=acc[:], in0=acc[:], in1=cs[:], op=Alu.add)
                    nc.sync.dma_start(d_out[:], acc[:, 0])

    nc.compile()
    return nc


def host_consts():
    lnsw = np.array(SWV, dtype=np.float32)[None, :]
    rowimg = np.zeros((len(SLABS), 128), dtype=np.float32)
    vmask = np.zeros((len(SLABS), 128), dtype=np.float32)
    for si, (base, nrows) in enumerate(SLABS):
        for p in range(128):
            sr = base + p
            if sr < TR:
                rowimg[si, p] = float(sr // PADH)
                vmask[si, p] = 1.0
    return lnsw, rowimg, vmask


_NC_CACHE = {}


def get_nc(repeat=1):
    if repeat not in _NC_CACHE:
        _NC_CACHE[repeat] = build_bass(repeat)
    return _NC_CACHE[repeat]


def make_in_maps(images, segmentations, ROIs, seg_label):
    lnsw, rowimg, vmask = host_consts()
    in_maps = []
    for c in range(NCORES):
        sl = slice(c * NIMG, (c + 1) * NIMG)
        in_maps.append({
            "images": np.ascontiguousarray(images[sl], dtype=np.float32),
            "segmentations": np.ascontiguousarray(segmentations[sl], dtype=np.float32),
            "ROIs": np.ascontiguousarray(ROIs[sl], dtype=np.float32),
            "seg_label": np.ascontiguousarray(seg_label[sl, 0], dtype=np.int32),
            "lnsw": lnsw, "rowimg": rowimg, "vmask": vmask,
        })
    return in_maps


def kernel(images, segmentations, ROIs, seg_label):
    from concourse.bass_utils import run_bass_kernel_spmd
    nc = get_nc()
    in_maps = make_in_maps(images, segmentations, ROIs, seg_label)
    res = run_bass_kernel_spmd(nc, in_maps, list(range(NCORES)))
    total = 0.0
    for c in range(NCORES):
        total += float(np.sum(res.results[c]["out"].astype(np.float64)))
    loss = np.float32(-WEIGHT * total / N)
    return np.reshape(loss, (1,))


if __name__ == "__main__":
    rng = np.random.default_rng(0)
    imgs = rng.uniform(0, 255, (N, C, H, W)).astype(np.float32)
    segs = rng.standard_normal((N, K, H, W)).astype(np.float32)
    e = np.exp(segs - segs.max(axis=1, keepdims=True))
    segs = (e / e.sum(axis=1, keepdims=True)).astype(np.float32)
    rois = rng.integers(0, 2, (N, H, W)).astype(np.float32)
    labs = rng.integers(0, 256, (N, 1, H, W)).astype(np.int32)
    print(kernel(images=imgs, segmentations=segs, ROIs=rois, seg_label=labs))
